# revision 1
# baseline (speedup 1.0000x reference)
"""AllPoleDigitalFilter Trainium2 kernel.

y[t] = K_int[t]*x[t] - sum_{i=1..30} a_int[t,i] * y[t-i]
with a_int/K_int linearly interpolated from frame coefficients (frame period 80).

Strategy (per core, 8 of 64 batch sequences):
 - Overlap-save chunking: each sequence split into 16 chunks of L=1000 samples;
   each chunk instance recomputes a W=120-sample warmup from zero state (the
   filter's homogeneous response decays below ~6e-6 within 120 samples for
   these coefficients: sum_i |a_i| <= 0.63).
 - 128 partitions = 128 chunk instances (8 seqs x 16 chunks). The order-30
   recurrence runs as one scalar_tensor_tensor (+accumulator read) per sample
   on the Vector engine:
     ybuf[p, 30+j] = sum_d A[p, j, d] * ybuf[p, j+d],  d in [0, 31)
   where A[p,j,d] = -a_int[t, 30-d] for d<30 and A[p,j,30] = K_int*x; ybuf
   slots not yet computed are prefilled with 1.0 so the last window element
   contributes the input term, and the accumulator result overwrites it.
 - The A coefficient stream (31 floats per sample) lives in one resident
   [128, 1120, 31] SBUF buffer. Interpolation splits across engines: a
   160-sample lead block is generated on the Vector engine (sized to cover
   the ScalarE stream latency before the chain reaches block 1), the
   per-sample fraction*delta term for the rest runs as 80 coarse ScalarE
   activation ops (per frame-position the fraction is a per-partition
   constant -> Copy with scale AP) fully hidden under the chain, and only
   the frame-term add remains in-chain on Vector. Half-frame coefficient
   tables arrive pre-gathered from the host (pure layout); outputs stream
   back in two slabs, the first mid-chain.
"""
import numpy as np

B, T = 64, 16000
NSEQ = 8           # sequences per core
NCORE = 8
W = 120            # warmup samples per chunk
L = 1000           # chunk payload
WP = W + L         # window samples per instance (1240)
NU = 32            # half-frame slots stored per partition
XP_LEN = W + T     # 16240

_prog = None


def _build_program():
    import concourse.bacc as bacc
    import concourse.mybir as mybir
    import concourse.bass as bass
    from concourse.tile import TileContext

    from concourse.tile import add_dep_helper
    f32 = mybir.dt.float32
    AP = bass.AP
    mult = mybir.AluOpType.mult
    add = mybir.AluOpType.add
    sub = mybir.AluOpType.subtract

    nc = bacc.Bacc("TRN2", target_bir_lowering=False, name="apdf",
                   detect_race_conditions=False)
    xp_d = nc.dram_tensor("xp", (NSEQ, XP_LEN), f32, kind="ExternalInput")
    frh_d = nc.dram_tensor("frh", (128, NU, 31), f32, kind="ExternalInput")
    frh1_d = nc.dram_tensor("frh1", (128, NU, 31), f32, kind="ExternalInput")
    ftab_d = nc.dram_tensor("ftabN", (128, WP), f32, kind="ExternalInput")
    ftabT_d = nc.dram_tensor("ftabT", (128, 80), f32, kind="ExternalInput")
    y_d = nc.dram_tensor("y", (NSEQ, T), f32, kind="ExternalOutput")

    # partition p = parity*64 + s*8 + k ; chunk m = 2*k + parity
    # window start w0 = 1000*m - W ; phase phi = 40*(1-parity)
    # base frame n0: parity 0: 25k - 2 (k=0 clamped to 0), parity 1: 25k + 11

    with TileContext(nc) as tc:
        with tc.tile_pool(name="sbuf", bufs=1) as pool:
            frh = pool.tile([128, NU, 31], f32)
            frh1 = pool.tile([128, NU, 31], f32)
            dfh = pool.tile([128, NU, 31], f32)
            frhN = pool.tile([128, NU, 31], f32)
            xwin = pool.tile([128, WP], f32)
            ybuf = pool.tile([128, 30 + WP], f32)
            ftab = pool.tile([128, WP], f32)
            ftabT = pool.tile([128, 80], f32)
            xgf = pool.tile([128, WP], f32)
            t2 = pool.tile([128, WP], f32)
            t3 = pool.tile([128, WP], f32)
            scr = pool.tile([128, 31], f32)
            afull = pool.tile([128, WP, 31], f32)

            # ---------------- input DMAs ----------------
            nc.sync.dma_start(out=ftab[:], in_=ftab_d[:])
            nc.sync.dma_start(out=ftabT[:], in_=ftabT_d[:])

            # half-frame coefficient tables, pre-arranged on host:
            # frh[p, u]  = a_frames[s(p), n0(p) + floor((40u+phi_p)/80)]
            # frh1[p, u] = same + 1 frame  (k=0 clamped; pure layout/gather)
            nc.sync.dma_start(out=frh[:].rearrange("p u d -> p (u d)"),
                              in_=frh_d[:].rearrange("p u d -> p (u d)"))
            nc.sync.dma_start(out=frh1[:].rearrange("p u d -> p (u d)"),
                              in_=frh1_d[:].rearrange("p u d -> p (u d)"))

            # x windows: partition (parity, s, k) <- xp[s, 1000*(2k+parity) : +WP]
            xw4 = xwin[:].rearrange("(c s k) j -> c s k j", c=2, s=8, k=8)
            for par in (0, 1):
                for s in range(NSEQ):
                    xsrc = AP(tensor=xp_d, offset=s * XP_LEN + 1000 * par,
                              ap=[[2000, 8], [1, WP]])
                    eng = nc.scalar if par == 0 else nc.gpsimd
                    eng.dma_start(out=xw4[par, s], in_=xsrc)

            nc.vector.tensor_tensor(
                out=dfh[:].rearrange("p u d -> p (u d)"),
                in0=frh1[:].rearrange("p u d -> p (u d)"),
                in1=frh[:].rearrange("p u d -> p (u d)"),
                op=sub,
            )
            nc.vector.tensor_scalar_mul(
                frhN[:, :, 0:30], frh[:, :, 30:0:-1], -1.0,
            )

            # xg for the whole window: Kint = K - ftab*dK ; xgf = Kint * xwin
            nc.vector.tensor_tensor(
                out=t2[:].rearrange("p (u r) -> p u r", r=40),
                in0=ftab[:].rearrange("p (u r) -> p u r", r=40),
                in1=dfh[:, 0 : WP // 40, 0][:, :, None].broadcast_to([128, WP // 40, 40]),
                op=mult,
            )
            nc.vector.tensor_tensor(
                out=t3[:].rearrange("p (u r) -> p u r", r=40),
                in0=frh[:, 0 : WP // 40, 0][:, :, None].broadcast_to([128, WP // 40, 40]),
                in1=t2[:].rearrange("p (u r) -> p u r", r=40),
                op=sub,
            )
            nc.vector.tensor_tensor(out=xgf[:], in0=t3[:], in1=xwin[:], op=mult)


            xg_copy = nc.scalar.activation(
                out=afull[:, :, 30], in_=xgf[:],
                func=mybir.ActivationFunctionType.Copy, bias=0.0, scale=1.0)


            def pass2(ts, j0, u0):
                nu_t = ts // 40
                av = afull[:, j0 : j0 + ts, 0:30].rearrange(
                    "p (u r) d -> p u r d", r=40)
                return nc.vector.tensor_tensor(
                    out=av,
                    in0=av,
                    in1=frhN[:, u0 : u0 + nu_t, None, 0:30].broadcast_to(
                        [128, nu_t, 40, 30]),
                    op=add,
                )

            # block 0 (fast start): both interp passes on DVE. Sized 160 so
            # its steps (~37us) still cover the ScalarE pass-1 stream latency
            # (80 samples measured too small, 240 larger than needed).
            av0 = afull[:, 0:160, 0:30].rearrange("p (u r) d -> p u r d", r=40)
            nc.vector.tensor_tensor(
                out=av0,
                in0=ftab[:, 0:160].rearrange("p (u r) -> p u r", r=40)
                    [:, :, :, None].broadcast_to([128, 4, 40, 30]),
                in1=dfh[:, 0:4, None, 30:0:-1].broadcast_to([128, 4, 40, 30]),
                op=mult,
            )
            pass2(160, 0, 0)


            # blocks 1+: interp pass 1 on ScalarE (own SBUF port, parallel
            # with the vector chain): for fixed frame position r the fraction
            # is a per-partition constant -> activation Copy with scale AP.
            # A[p, 80q + r, d] = ftabT[p, r] * dfh[p, 6 + 2q + (r>=40), 30-d]
            act_last = None
            for r in range(80):
                off = 1 if r >= 40 else 0
                act_last = nc.scalar.activation(
                    out=afull[:, 160 + r : WP : 80, 0:30],
                    in_=dfh[:, 4 + off : 4 + off + 2 * 12 : 2, 30:0:-1],
                    func=mybir.ActivationFunctionType.Copy,
                    bias=0.0,
                    scale=ftabT[:, r : r + 1],
                )


            # generate tile 0 coefficients first (chain can start while the
            # x-window DMAs for the xg pass are still landing)

            # ---------------- y buffer init ----------------
            nc.gpsimd.memset(ybuf[:, 0:30], 0.0)
            nc.gpsimd.memset(ybuf[:, 30:], 1.0)

            # xg column for the whole window (ScalarE, parallel)
            # ------------- stepping + in-chain pass2 (vector) ----
            BLOCKS = [160, 240, 240, 240, 240]
            j0 = 0
            u0 = 0
            for bi, ts in enumerate(BLOCKS):
                if bi >= 1:
                    p2 = pass2(ts, j0, u0)
                    add_dep_helper(p2.ins, act_last.ins, sync=True,
                                   reason="pass2 reads ScalarE pass1 output")
                first_step = True
                for jl in range(ts):
                    j = j0 + jl
                    st = nc.vector.scalar_tensor_tensor(
                        out=scr[:],
                        in0=afull[:, j, :],
                        scalar=0.0,
                        in1=ybuf[:, j : j + 31],
                        op0=mybir.AluOpType.bypass,
                        op1=mult,
                        accum_out=ybuf[:, 30 + j : 31 + j],
                    )
                    if first_step:
                        add_dep_helper(st.ins, xg_copy.ins, sync=True,
                                       reason="steps read xg column")
                        if bi >= 1:
                            add_dep_helper(st.ins, act_last.ins, sync=True,
                                           reason="steps read ScalarE pass1 output")
                        first_step = False
                j0 += ts
                u0 += ts // 40

                if j0 == 640:
                    yva = ybuf[:, 30 + W : 30 + W + 500].rearrange(
                        "(c s k) j -> c s k j", c=2, s=8, k=8)
                    for par in (0, 1):
                        for s in range(NSEQ):
                            dst = AP(tensor=y_d, offset=s * T + 1000 * par,
                                     ap=[[2000, 8], [1, 500]])
                            eng = nc.scalar if (s % 2 == 0) else nc.sync
                            eng.dma_start(out=dst, in_=yva[par, s])

            # ---------------- output DMAs ----------------
            yv = ybuf[:, 30 + W + 500 : 30 + W + L].rearrange(
                "(c s k) j -> c s k j", c=2, s=8, k=8)
            for par in (0, 1):
                for s in range(NSEQ):
                    dst = AP(tensor=y_d, offset=s * T + 1000 * par + 500,
                             ap=[[2000, 8], [1, 500]])
                    eng = nc.scalar if (s % 2 == 0) else nc.sync
                    eng.dma_start(out=dst, in_=yv[par, s])

    nc.compile()
    return nc


def _get_prog():
    global _prog
    if _prog is None:
        _prog = _build_program()
    return _prog


def _host_inputs(x, a):
    x = np.ascontiguousarray(x, dtype=np.float32)
    a = np.ascontiguousarray(a, dtype=np.float32)
    xp = np.zeros((B, XP_LEN), np.float32)
    xp[:, W:] = x
    # replicate-padded frames per sequence: [B, 203, 31]
    af = np.concatenate([a, a[:, -1:, :], np.zeros((B, 1, 31), np.float32)], axis=1)
    # per-partition half-frame tables (pure gather): p = parity*64 + s*8 + k,
    # chunk m = 2k + parity, w0 = 1000m - W, phi = w0 mod 80,
    # n0 = floor(w0/80) (clamped at 0 for m=0)
    par = np.arange(128) // 64
    sq = (np.arange(128) % 64) // 8
    k = np.arange(128) % 8
    m = 2 * k + par
    w0 = 1000 * m - W
    n0 = np.floor_divide(w0, 80)
    phi = w0 - 80 * n0
    u = np.arange(NU)
    nl = (40 * u[None, :] + phi[:, None]) // 80          # [128, NU]
    idx = np.clip(n0[:, None] + nl, 0, af.shape[1] - 1)
    idx1 = np.clip(n0[:, None] + nl + 1, 0, af.shape[1] - 1)
    jl = np.arange(WP)
    ftabN = -(((jl[None, :] + phi[:, None]) % 80) / 80.0).astype(np.float32)
    rr = np.arange(80)
    ftabT = -(((rr[None, :] + phi[:, None]) % 80) / 80.0).astype(np.float32)
    in_maps = []
    for c in range(NCORE):
        sl = slice(c * NSEQ, (c + 1) * NSEQ)
        in_maps.append({
            "xp": xp[sl],
            "frh": af[c * NSEQ + sq[:, None], idx].astype(np.float32),
            "frh1": af[c * NSEQ + sq[:, None], idx1].astype(np.float32),
            "ftabN": ftabN.astype(np.float32),
            "ftabT": ftabT,
        })
    return in_maps


def kernel(x, a):
    from concourse import bass_utils

    nc = _get_prog()
    in_maps = _host_inputs(x, a)
    res = bass_utils.run_bass_kernel_spmd(nc, in_maps, core_ids=list(range(NCORE)))
    out = np.empty((B, T), np.float32)
    for c in range(NCORE):
        out[c * NSEQ : (c + 1) * NSEQ] = res.results[c]["y"]
    return out



# revision 2
# speedup vs baseline: 1.9165x; 1.9165x over previous
"""AllPoleDigitalFilter Trainium2 kernel — segmented block-solve version.

y[t] = K_int[t]*x[t] - sum_{i=1..30} a_int[t,i] * y[t-i]
with a_int/K_int linearly interpolated from frame coefficients (period 80).

Strategy (per core, 8 of 64 batch sequences):
 - Overlap-save chunking: 16 chunks of L=1000 per sequence, W=80-sample
   warmup recomputed from zero state (homogeneous response decays to
   ~3e-4 of initial magnitude in 80 samples for these coefficients).
 - 128 partitions = 128 chunk instances (2 parities x 8 seqs x 8 chunks),
   window = 1080 samples each.
 - The order-30 recurrence advances S=8 samples per 5 Vector-engine
   instructions (instead of 2 instructions per sample):
     prod1[k,d] = afull[t0+k, d] * ybuf[t0+k+d]   (d=0..29, lag 30-d;
                  in-segment ybuf slots are still zero, so in-segment
                  taps contribute nothing)
     F[k]  = reduce_X(prod1)                       (known-history part)
     b[k]  = xgf[t0+k] - F[k]
     prod2 = E[seg] * b  (broadcast)               (8x8 matvec)
     y_seg = reduce_X(prod2) -> ybuf[30+t0 : +8]
   where E[seg] = (I + N_seg)^{-1}, N_seg the strictly-lower in-segment
   coefficient matrix; E is precomputed on-device per segment via a
   7-step forward substitution (rect-multiply + negated reduce).
 - Coefficient interpolation runs OFF the Vector engine: the Pool engine
   computes frac*dfh with double-broadcast APs, and the frame term
   arrives via gpsimd accumulate-DMA from a host-side replicated gather
   (pure layout). The gain channel xgf is assembled the same way.
"""
import numpy as np

B, T = 64, 16000
NSEQ = 8           # sequences per core
NCORE = 8
W = 80             # warmup samples per chunk
L = 1000           # chunk payload
WP = W + L         # window samples per instance (1080)
S = 8              # segment length
NSEG = WP // S     # 135
NU = WP // 40      # 27 half-frames per window
XP_LEN = W + T     # 16080

# afull-assembly blocks in half-frames (must sum to NU)
BLK_U = [5, 5, 5, 5, 5, 2]

_prog = None


def _build_program():
    import concourse.bacc as bacc
    import concourse.mybir as mybir
    import concourse.bass as bass
    from concourse.tile import TileContext

    f32 = mybir.dt.float32
    AP = bass.AP
    mult = mybir.AluOpType.mult
    add = mybir.AluOpType.add
    sub = mybir.AluOpType.subtract
    AXX = mybir.AxisListType.X

    nc = bacc.Bacc("TRN2", target_bir_lowering=False, name="apdf2",
                   detect_race_conditions=False)
    xp_d = nc.dram_tensor("xp", (NSEQ, XP_LEN), f32, kind="ExternalInput")
    frhr_d = nc.dram_tensor("frhr", (128, NU, 30), f32, kind="ExternalInput")
    frh1r_d = nc.dram_tensor("frh1r", (128, NU, 30), f32, kind="ExternalInput")
    kfr_d = nc.dram_tensor("kfr", (128, NU), f32, kind="ExternalInput")
    kfr1_d = nc.dram_tensor("kfr1", (128, NU), f32, kind="ExternalInput")
    ftab_d = nc.dram_tensor("ftab", (128, WP), f32, kind="ExternalInput")
    frhrep_d = nc.dram_tensor("frhrep", (128, WP, 30), f32, kind="ExternalInput")
    krep_d = nc.dram_tensor("krep", (128, WP), f32, kind="ExternalInput")
    y_d = nc.dram_tensor("y", (NSEQ, T), f32, kind="ExternalOutput")

    with TileContext(nc) as tc:
        with tc.tile_pool(name="sbuf", bufs=1) as pool:
            afull = pool.tile([128, WP, 30], f32)      # 129.6 KB/part
            E = pool.tile([128, NSEG, 64], f32)        # 34.6 KB
            escr = pool.tile([128, 25 * 49], f32)      # 4.9 KB
            ybuf = pool.tile([128, 30 + WP], f32)      # 4.44 KB
            xwin = pool.tile([128, WP], f32)
            xgf = pool.tile([128, WP], f32)
            kt = pool.tile([128, WP], f32)
            ftab = pool.tile([128, WP], f32)
            frhr = pool.tile([128, NU, 30], f32)
            frh1r = pool.tile([128, NU, 30], f32)
            dfhr = pool.tile([128, NU, 30], f32)
            kfr = pool.tile([128, NU], f32)
            kfr1 = pool.tile([128, NU], f32)
            dk = pool.tile([128, NU], f32)
            prod1 = pool.tile([128, S, 30], f32)
            prod2 = pool.tile([128, S, S], f32)
            fred = pool.tile([128, S], f32)
            bseg = pool.tile([128, S], f32)

            def tap(t, ap):
                """Manual AP over a tile's storage (element strides)."""
                base = t[:]
                return AP(tensor=base.tensor, offset=ap[0], ap=ap[1])

            AF = WP * 30     # afull partition stride
            ES = NSEG * 64   # E partition stride

            # ---------------- input DMAs ----------------
            nc.sync.dma_start(out=ftab[:], in_=ftab_d[:])
            nc.sync.dma_start(out=frhr[:].rearrange("p u d -> p (u d)"),
                              in_=frhr_d[:].rearrange("p u d -> p (u d)"))
            nc.sync.dma_start(out=frh1r[:].rearrange("p u d -> p (u d)"),
                              in_=frh1r_d[:].rearrange("p u d -> p (u d)"))
            nc.scalar.dma_start(out=kfr[:], in_=kfr_d[:])
            nc.scalar.dma_start(out=kfr1[:], in_=kfr1_d[:])

            # x windows: partition (parity, s, k) <- xp[s, 1000*(2k+par) : +WP]
            xw4 = xwin[:].rearrange("(c s k) j -> c s k j", c=2, s=8, k=8)
            for par in (0, 1):
                for s in range(NSEQ):
                    xsrc = AP(tensor=xp_d, offset=s * XP_LEN + 1000 * par,
                              ap=[[2000, 8], [1, WP]])
                    eng = nc.scalar if par == 0 else nc.sync
                    eng.dma_start(out=xw4[par, s], in_=xsrc)

            # ---------------- init ----------------
            nc.gpsimd.memset(ybuf[:], 0.0)
            nc.gpsimd.memset(E[:].rearrange("p s e -> p (s e)"), 0.0)
            # E diagonal = 1
            nc.gpsimd.memset(
                tap(E, (0, [[ES, 128], [64, NSEG], [9, 8]])), 1.0)

            # deltas
            nc.vector.tensor_tensor(
                out=dfhr[:].rearrange("p u d -> p (u d)"),
                in0=frh1r[:].rearrange("p u d -> p (u d)"),
                in1=frhr[:].rearrange("p u d -> p (u d)"), op=sub)
            nc.vector.tensor_tensor(out=dk[:], in0=kfr1[:], in1=kfr[:], op=sub)

            # ---------------- per-block pipeline ----------------
            u0 = 0
            s0 = 0
            out_slab = 0
            for blki, ublk in enumerate(BLK_U):
                c0 = u0 * 40
                cn = ublk * 40          # samples in block
                segb = cn // S          # segments in block

                # interp: afull[:, c0:c0+cn, :] = dfhr(u) * ftab(j)
                nc.gpsimd.tensor_tensor(
                    out=tap(afull, (c0 * 30,
                            [[AF, 128], [1200, ublk], [30, 40], [1, 30]])),
                    in0=tap(dfhr, (u0 * 30,
                            [[NU * 30, 128], [30, ublk], [0, 40], [1, 30]])),
                    in1=tap(ftab, (c0,
                            [[WP, 128], [40, ublk], [1, 40], [0, 30]])),
                    op=mult)
                # += frame term (replicated gather from DRAM)
                nc.gpsimd.dma_start(
                    out=tap(afull, (c0 * 30, [[AF, 128], [1, cn * 30]])),
                    in_=AP(tensor=frhrep_d, offset=c0 * 30,
                           ap=[[AF, 128], [1, cn * 30]]),
                    accum_op=add)

                # gain channel for this block: kt = dk(u)*ftab; kt += krep;
                # xgf = kt * xwin
                nc.gpsimd.tensor_tensor(
                    out=tap(kt, (c0, [[WP, 128], [40, ublk], [1, 40]])),
                    in0=tap(dk, (u0, [[NU, 128], [1, ublk], [0, 40]])),
                    in1=tap(ftab, (c0, [[WP, 128], [40, ublk], [1, 40]])),
                    op=mult)
                nc.gpsimd.dma_start(
                    out=tap(kt, (c0, [[WP, 128], [1, cn]])),
                    in_=AP(tensor=krep_d, offset=c0, ap=[[WP, 128], [1, cn]]),
                    accum_op=add)
                nc.gpsimd.tensor_tensor(
                    out=tap(xgf, (c0, [[WP, 128], [1, cn]])),
                    in0=tap(kt, (c0, [[WP, 128], [1, cn]])),
                    in1=tap(xwin, (c0, [[WP, 128], [1, cn]])),
                    op=mult)

                # ---- E precompute for this block's segments (DVE) ----
                for kk in range(1, S):
                    # escr[p, sb, j, i] = a[t0+kk, lag i] * E[sb, kk-i, j]
                    nc.vector.tensor_tensor(
                        out=tap(escr, (0,
                                [[25 * 49, 128], [kk * kk, segb],
                                 [kk, kk], [1, kk]])),
                        in0=tap(afull, ((s0 * S + kk) * 30 + 29,
                                [[AF, 128], [240, segb], [0, kk], [-1, kk]])),
                        in1=tap(E, (s0 * 64 + (kk - 1) * 8,
                                [[ES, 128], [64, segb], [1, kk], [-8, kk]])),
                        op=mult)
                    # E[sb, kk, 0:kk] = -sum_i escr
                    nc.vector.tensor_reduce(
                        out=tap(E, (s0 * 64 + kk * 8,
                                [[ES, 128], [64, segb], [1, kk]])),
                        in_=tap(escr, (0,
                                [[25 * 49, 128], [kk * kk, segb],
                                 [kk, kk], [1, kk]])),
                        axis=AXX, op=add, negate=True)

                # ---- chain segments (DVE) ----
                for sl in range(segb):
                    seg = s0 + sl
                    t0 = seg * S
                    nc.vector.tensor_tensor(
                        out=prod1[:],
                        in0=afull[:, t0 : t0 + S, :],
                        in1=tap(ybuf, (t0, [[30 + WP, 128], [1, S], [1, 30]])),
                        op=mult)
                    nc.vector.tensor_reduce(
                        out=fred[:], in_=prod1[:], axis=AXX, op=add)
                    nc.vector.tensor_tensor(
                        out=bseg[:], in0=xgf[:, t0 : t0 + S], in1=fred[:],
                        op=sub)
                    nc.vector.tensor_tensor(
                        out=prod2[:],
                        in0=tap(E, (seg * 64, [[ES, 128], [8, 8], [1, 8]])),
                        in1=tap(bseg, (0, [[S, 128], [0, 8], [1, 8]])),
                        op=mult)
                    nc.vector.tensor_reduce(
                        out=ybuf[:, 30 + t0 : 30 + t0 + S],
                        in_=prod2[:].rearrange("p a b -> p a b"),
                        axis=AXX, op=add)

                u0 += ublk
                s0 += segb

                # first output slab once payload [0,500) is done (after
                # block 3: samples 0..800 cover payload up to 720)
                if blki == 3 and out_slab == 0:
                    out_slab = 1
                    yva = ybuf[:, 30 + W : 30 + W + 500].rearrange(
                        "(c s k) j -> c s k j", c=2, s=8, k=8)
                    for par in (0, 1):
                        for s in range(NSEQ):
                            dst = AP(tensor=y_d, offset=s * T + 1000 * par,
                                     ap=[[2000, 8], [1, 500]])
                            eng = nc.scalar if (s % 2 == 0) else nc.sync
                            eng.dma_start(out=dst, in_=yva[par, s])

            # ---------------- final output DMAs ----------------
            yv = ybuf[:, 30 + W + 500 : 30 + W + L].rearrange(
                "(c s k) j -> c s k j", c=2, s=8, k=8)
            for par in (0, 1):
                for s in range(NSEQ):
                    dst = AP(tensor=y_d, offset=s * T + 1000 * par + 500,
                             ap=[[2000, 8], [1, 500]])
                    eng = nc.scalar if (s % 2 == 0) else nc.sync
                    eng.dma_start(out=dst, in_=yv[par, s])

    nc.compile()
    return nc


def _get_prog():
    global _prog
    if _prog is None:
        _prog = _build_program()
    return _prog


def _host_inputs(x, a):
    x = np.ascontiguousarray(x, dtype=np.float32)
    a = np.ascontiguousarray(a, dtype=np.float32)
    xp = np.zeros((B, XP_LEN), np.float32)
    xp[:, W:] = x
    # replicate-padded frames per sequence: [B, 201, 31]
    af = np.concatenate([a, a[:, -1:, :]], axis=1)
    nfr = af.shape[1]  # 201
    # partition p = parity*64 + s*8 + k ; chunk m = 2k + parity
    par = np.arange(128) // 64
    sq = (np.arange(128) % 64) // 8
    kc = np.arange(128) % 8
    m = 2 * kc + par
    w0 = 1000 * m - W
    n0 = np.floor_divide(w0, 80)
    phi = w0 - 80 * n0              # 0 or 40
    u = np.arange(NU)
    nl = (40 * u[None, :] + phi[:, None]) // 80          # [128, NU]
    idx = np.clip(n0[:, None] + nl, 0, nfr - 1)
    idx1 = np.clip(n0[:, None] + nl + 1, 0, nfr - 1)
    jl = np.arange(WP)
    ftab = (((jl[None, :] + phi[:, None]) % 80) / 80.0).astype(np.float32)
    rev = 30 - np.arange(30)        # d -> coeff index 30-d (lag 30-d)
    in_maps = []
    for c in range(NCORE):
        sl = slice(c * NSEQ, (c + 1) * NSEQ)
        seqg = c * NSEQ + sq
        frhr = af[seqg[:, None, None], idx[:, :, None], rev[None, None, :]]
        frh1r = af[seqg[:, None, None], idx1[:, :, None], rev[None, None, :]]
        kfr = af[seqg[:, None], idx, 0]
        kfr1 = af[seqg[:, None], idx1, 0]
        in_maps.append({
            "xp": xp[sl],
            "frhr": np.ascontiguousarray(frhr, np.float32),
            "frh1r": np.ascontiguousarray(frh1r, np.float32),
            "kfr": np.ascontiguousarray(kfr, np.float32),
            "kfr1": np.ascontiguousarray(kfr1, np.float32),
            "ftab": ftab,
            "frhrep": np.ascontiguousarray(
                np.repeat(frhr, 40, axis=1), np.float32),
            "krep": np.ascontiguousarray(
                np.repeat(kfr, 40, axis=1), np.float32),
        })
    return in_maps


def kernel(x, a):
    from concourse import bass_utils

    nc = _get_prog()
    in_maps = _host_inputs(x, a)
    res = bass_utils.run_bass_kernel_spmd(nc, in_maps, core_ids=list(range(NCORE)))
    out = np.empty((B, T), np.float32)
    for c in range(NCORE):
        out[c * NSEQ : (c + 1) * NSEQ] = res.results[c]["y"]
    return out
